# revision 5
# baseline (speedup 1.0000x reference)
"""CrossAttention Trainium2 kernel, fp8 DoubleRow edition (1 batch elem/core).

Math per batch element (C=256 channels, N=3136 positions):
    q = Wq xq, k = Wk xkv, pv = (Wproj Wv) xkv          (1x1 convs)
    S[n,m] = q[:,n].k[:,m];  rd[m] = 1/(||q[:,m]|| ||k[:,m]|| + eps)
    A = softmax(S * rd[m] along m);  out = Wproj(A v) + xq residual

Implementation notes:
  * All matmuls in fp8e4 DoubleRow mode (2 fp8 rows/cycle, K=256 in one op).
    Weights and x are scaled by 16 on the host so w entries ~N(0,1) stay out
    of fp8 subnormals. Scales self-cancel in softmax: q,k scale 16 each =>
    S x256, qn2*kn2 x65536, rd = rsqrt(qn2*kn2) absorbs it. pv scale 16 is
    divided out in the output STT.
  * S^T tiles [m=128, n<=1024] (2 psum banks) -> ONE ACT Exp per (chunk,
    n-super) with per-partition scale rd[:,mi], writing fp8 er directly.
    |S*rd| <= ~1.1 so no max-subtraction is needed (exp in [0.3, 3.1]).
  * Norms via transposed DR projections (qT [m,256] psum) + DVE square +
    X-reduce; rd = rsqrt via DVE bit-trick + 2 Newton steps (ACT Sqrt would
    thrash the Exp activation table; gpsimd cannot read PSUM).
  * AV: er (stationary) x pvT|1,1 (moving) accumulating [n-sub, C+2]; the
    ones channels give the softmax row-sum for free. Normalize on DVE,
    bf16 PE transpose back to [c, n], fused (tp/16 + xq) STT, DMA out.
  * PSUM: S pool 2x2 banks + aux pool 4x1 bank (phase-1 qT/kT/pv/proj tiles,
    then AV acc + transpose) = 8 banks exactly.
"""

import sys

if "/opt/trn_rl_repo" not in sys.path:
    sys.path.insert(0, "/opt/trn_rl_repo")

import numpy as np
import ml_dtypes

import concourse.bass as bass
import concourse.mybir as mybir
import concourse.tile as tile
from concourse import bacc
from concourse.bass_utils import run_bass_kernel_spmd
from concourse.masks import make_identity
from contextlib import ExitStack

F32 = mybir.dt.float32
F8 = mybir.dt.float8e4
BF16 = mybir.dt.bfloat16
I32 = mybir.dt.int32
AF = mybir.ActivationFunctionType
DR = mybir.MatmulPerfMode.DoubleRow
ALU = mybir.AluOpType

P = 128
C = 256
CC = C // P            # 2
N = 56 * 56            # 3136
SEG = 512
SEGS = [(i, min(SEG, N - i)) for i in range(0, N, SEG)]            # 7
M_CHUNKS = [(i, min(P, N - i)) for i in range(0, N, P)]            # 25
N_FULL = len(M_CHUNKS) - 1                                         # 24 full
SUPERS = [(0, 1024), (1024, 1024), (2048, 1024), (3072, 64)]
WSCALE = 16.0          # host-side weight/x scale for fp8 range
# pv is needed only once AV starts (after the last seg): keep its psum
# traffic and DVE casts out of the phase-1 production pipeline entirely
PV_PLAN = {6: range(0, 25)}


def _chunks_of_seg(si):
    lo = si * 4
    return [(mi, M_CHUNKS[mi][0], M_CHUNKS[mi][1])
            for mi in range(lo, min(lo + 4, len(M_CHUNKS)))]


def build(use_bias: bool):
    nc = bacc.Bacc(None, target_bir_lowering=False)

    xq_d = nc.dram_tensor("xq", [C, N], BF16, kind="ExternalInput")
    x8q_d = nc.dram_tensor("x8q", [C, N], F8, kind="ExternalInput")
    xkv8_d = nc.dram_tensor("xkv8", [C, N], F8, kind="ExternalInput")
    wq8_d = nc.dram_tensor("wq8", [C, C], F8, kind="ExternalInput")   # [c, d]
    wk8_d = nc.dram_tensor("wk8", [C, C], F8, kind="ExternalInput")
    w38_d = nc.dram_tensor("w38", [C, C], F8, kind="ExternalInput")
    gw8_d = nc.dram_tensor("gw8", [C, C], F8, kind="ExternalInput")   # 16*Wq.T@Wk
    bq_d = nc.dram_tensor("bq16", [C], F32, kind="ExternalInput")     # 16*bq
    bk_d = nc.dram_tensor("bk16", [C], F32, kind="ExternalInput")     # 16*bk
    bo_d = nc.dram_tensor("bo", [C], F32, kind="ExternalInput")       # Wproj bv + bproj
    out_d = nc.dram_tensor("out", [C, N], BF16, kind="ExternalOutput")

    xq_v = xq_d[:].rearrange("(cc p) n -> p cc n", p=P)
    x8q_v = x8q_d[:].rearrange("(cc p) n -> p cc n", p=P)
    xkv8_v = xkv8_d[:].rearrange("(cc p) n -> p cc n", p=P)
    out_v = out_d[:].rearrange("(cc p) n -> p cc n", p=P)

    with tile.TileContext(nc) as tc, ExitStack() as ctx:
        pers = ctx.enter_context(tc.tile_pool(name="pers", bufs=1))
        sS = ctx.enter_context(tc.tile_pool(name="sS", bufs=2, space="PSUM"))
        aux = ctx.enter_context(tc.tile_pool(name="aux", bufs=4, space="PSUM"))
        scrp = ctx.enter_context(tc.tile_pool(name="scrp", bufs=3))
        e8p = ctx.enter_context(tc.tile_pool(name="e8p", bufs=39))
        e8s = ctx.enter_context(tc.tile_pool(name="e8s", bufs=13))
        outp = ctx.enter_context(tc.tile_pool(name="outp", bufs=4))
        smls = ctx.enter_context(tc.tile_pool(name="smls", bufs=4))

        xq_f = pers.tile([P, CC, N], BF16)
        x8q = pers.tile([P, CC, N], F8)
        x8kv = pers.tile([P, CC, N], F8)
        if use_bias:
            q8 = pers.tile([P, CC, N], F8)
            k8 = pers.tile([P, CC, N], F8)
        else:
            # S = xkv^T (Wk^T Wq) xq: one fused projection g = G xq replaces
            # both q and k projections for the S matmul (norms still use
            # the transposed qT/kT products)
            g8 = pers.tile([P, CC, N], F8)
            gw8 = pers.tile([P, CC, C], F8)
        wq8 = pers.tile([P, CC, C], F8)
        wk8 = pers.tile([P, CC, C], F8)
        w38 = pers.tile([P, CC, C], F8)
        pvT8 = pers.tile([P, 13, 2, C + 2], F8)
        rd = pers.tile([P, len(M_CHUNKS)], F32)
        if use_bias:
            qn2 = pers.tile([P, len(M_CHUNKS)], F32)
            kn2 = pers.tile([P, len(M_CHUNKS)], F32)
        ident = pers.tile([P, P], BF16)
        if use_bias:
            bq_sb = pers.tile([P, CC], F32)
            bk_sb = pers.tile([P, CC], F32)
            bo_sb = pers.tile([P, CC], F32)
            bqb = pers.tile([P, C], F32)
            bkb = pers.tile([P, C], F32)

        def preamble():
            make_identity(nc, ident)
            nc.sync.dma_start(wq8, wq8_d[:].rearrange("(cc p) d -> p cc d", p=P))
            nc.sync.dma_start(wk8, wk8_d[:].rearrange("(cc p) d -> p cc d", p=P))
            nc.sync.dma_start(w38, w38_d[:].rearrange("(cc p) d -> p cc d", p=P))
            if not use_bias:
                nc.sync.dma_start(gw8,
                                  gw8_d[:].rearrange("(cc p) d -> p cc d", p=P))
            # softmax row-sum channels
            nc.vector.memset(pvT8[:, :, :, C : C + 2], 1.0)
        if use_bias:
            nc.sync.dma_start(bq_sb, bq_d[:].rearrange("(c p) -> p c", p=P))
            nc.sync.dma_start(bk_sb, bk_d[:].rearrange("(c p) -> p c", p=P))
            nc.sync.dma_start(bo_sb, bo_d[:].rearrange("(c p) -> p c", p=P))
            nc.sync.dma_start(
                bqb, bass.AP(tensor=bq_d[:].tensor, offset=0, ap=[[0, P], [1, C]])
            )
            nc.sync.dma_start(
                bkb, bass.AP(tensor=bk_d[:].tensor, offset=0, ap=[[0, P], [1, C]])
            )

        # ---------------- phase 1 (per 512-seg, pipelined) ----------------
        def norms_for_chunk(mi, m0, mw, ci, bag):
            for side, (which, xsrc, w8) in enumerate(
                (("q", x8q, wq8), ("k", x8kv, wk8))
            ):
                ps = aux.tile([P, SEG], F32, tag="aux", name=f"t{which}{m0}")
                nc.tensor.matmul(ps[:mw, :C], xsrc[:, :, m0 : m0 + mw], w8,
                                 start=True, stop=True, perf_mode=DR)
                if use_bias:
                    nacc = qn2 if which == "q" else kn2
                    bbt = bqb if which == "q" else bkb
                    scr = scrp.tile([P, C], F32, tag="sq", name=f"s{which}{m0}")
                    nc.vector.tensor_add(scr[:mw], ps[:mw, :C], bbt[:mw])
                    nc.vector.scalar_tensor_tensor(
                        scr[:mw], scr[:mw], 1.0, scr[:mw], ALU.mult, ALU.mult,
                        accum_out=nacc[:mw, mi : mi + 1])
                else:
                    # sum(x^2) = n*(var + mean^2) via bn_stats: one DVE pass
                    # over the PSUM tile instead of copy+square+reduce
                    bn6 = scrp.tile([P, 2, 6], F32, tag="bn6",
                                    name=f"b{which}{m0}")
                    nc.vector.bn_stats(bn6[:mw, side, :], ps[:mw, :C])
                    nc.vector.bn_aggr(bag[:mw, ci, side, :], bn6[:mw, side, :])

        def pv_for_chunk(mi, m0, mw):
            ps = aux.tile([P, SEG], F32, tag="aux", name=f"pv{m0}")
            nc.tensor.matmul(ps[:mw, :C], x8kv[:, :, m0 : m0 + mw], w38,
                             start=True, stop=True, perf_mode=DR)
            nc.vector.tensor_copy(pvT8[:mw, mi // 2, mi % 2, :C], ps[:mw, :C])

        def rd_for_seg(si, bag, part=None):
            lo = si * 4
            hi = min(lo + 4, len(M_CHUNKS))
            nseg = hi - lo
            c0_, c1_ = part if part else (0, nseg)
            u = smls.tile([P, 4], F32, tag="u", name=f"u{si}_{c0_}")
            sl = slice(c0_, c1_)
            if use_bias:
                nc.gpsimd.tensor_mul(u[:, sl], qn2[:, lo + c0_ : lo + c1_],
                                     kn2[:, lo + c0_ : lo + c1_])
            else:
                t = smls.tile([P, 4, 2, 1], F32, tag="tvm",
                              name=f"tvm{si}_{c0_}")
                mean = bag[:, sl, :, 0:1]
                var = bag[:, sl, :, 1:2]
                nc.gpsimd.tensor_mul(t[:, sl], mean, mean)
                nc.gpsimd.tensor_add(t[:, sl], t[:, sl], var)
                nc.gpsimd.tensor_mul(u[:, sl], t[:, sl, 0, 0],
                                     t[:, sl, 1, 0])
            yb = smls.tile([P, 4], I32, tag="yb", name=f"yb{si}_{c0_}")
            nc.vector.tensor_scalar(yb[:, sl], u[:, sl].bitcast(I32),
                                    1, None, ALU.logical_shift_right)
            nc.vector.tensor_scalar(yb[:, sl], yb[:, sl], -1, 0x5F3759DF,
                                    ALU.mult, ALU.add)
            y = yb.bitcast(F32)
            h = smls.tile([P, 4], F32, tag="h", name=f"h{si}_{c0_}")
            for _ in range(2):
                nc.gpsimd.tensor_mul(h[:, sl], y[:, sl], y[:, sl])
                nc.gpsimd.tensor_mul(h[:, sl], h[:, sl], u[:, sl])
                nc.vector.tensor_scalar(h[:, sl], h[:, sl], -0.5, 1.5,
                                        ALU.mult, ALU.add)
                nc.gpsimd.tensor_mul(y[:, sl], y[:, sl], h[:, sl])
            if use_bias:
                nc.gpsimd.tensor_copy(rd[:, lo + c0_ : lo + c1_], y[:, sl])
            else:
                # g-fold S is 16*S; u = (qn*kn/256)^2 => rd = rsqrt(u)/16
                nc.vector.tensor_scalar(rd[:, lo + c0_ : lo + c1_], y[:, sl],
                                        1.0 / 16.0, None, ALU.mult)

        def proj_for_seg(n0, nw, on_act=False):
            if use_bias:
                plan = (("q", x8q, wq8, q8, bq_sb), ("k", x8kv, wk8, k8, bk_sb))
            else:
                plan = (("g", x8q, gw8, g8, None),)
            for which, xsrc, w8, dst, bt in plan:
                for dc in range(CC):
                    ps = aux.tile([P, SEG], F32, tag="aux",
                                  name=f"p{which}{n0}_{dc}")
                    nc.tensor.matmul(ps[:, :nw], w8[:, :, dc * P : (dc + 1) * P],
                                     xsrc[:, :, n0 : n0 + nw],
                                     start=True, stop=True, perf_mode=DR)
                    if use_bias:
                        nc.vector.tensor_scalar(dst[:, dc, n0 : n0 + nw],
                                                ps[:, :nw], bt[:, dc : dc + 1],
                                                None, ALU.add)
                    elif on_act:
                        # lead-in: ACT Copy shares the Exp table and shortens
                        # the DVE dep chain in front of the first exps
                        nc.scalar.activation(dst[:, dc, n0 : n0 + nw],
                                             ps[:, :nw], AF.Copy)
                    else:
                        nc.vector.tensor_copy(dst[:, dc, n0 : n0 + nw],
                                              ps[:, :nw])

        def dma_seg(si):
            n0, nw = SEGS[si]
            nc.sync.dma_start(x8q[:, :, n0 : n0 + nw],
                              x8q_v[:, :, n0 : n0 + nw])
            nc.sync.dma_start(x8kv[:, :, n0 : n0 + nw],
                              xkv8_v[:, :, n0 : n0 + nw])
            nc.sync.dma_start(xq_f[:, :, n0 : n0 + nw],
                              xq_v[:, :, n0 : n0 + nw])

        def norms_seg(si):
            bag = smls.tile([P, 4, 2, 2], F32, tag="bag", name=f"bag{si}")
            for ci, (mi, m0, mw) in enumerate(_chunks_of_seg(si)):
                norms_for_chunk(mi, m0, mw, ci, bag)
                if si == 0 and ci == 0:
                    # chunk 0's rd alone unblocks the very first exp
                    rd_for_seg(0, bag, part=(0, 1))
            if si == 0:
                rd_for_seg(si, bag, part=(1, 4))
            else:
                rd_for_seg(si, bag)

        # ---------------- phase 2 ----------------
        er_tiles = {}

        def s_exp_chunk(sj, mi):
            sn0, snw = SUPERS[sj]
            m0, mw = M_CHUNKS[mi]
            sp = sS.tile([P, 2, SEG], F32, tag="sp", name=f"sp{sj}_{mi}")
            lhsT = k8 if use_bias else x8kv
            rhs = q8 if use_bias else g8
            nh = (snw + SEG - 1) // SEG
            for hi in range(nh):
                hw = min(SEG, snw - hi * SEG)
                nc.tensor.matmul(sp[:mw, hi, :hw], lhsT[:, :, m0 : m0 + mw],
                                 rhs[:, :, sn0 + hi * SEG : sn0 + hi * SEG + hw],
                                 start=True, stop=True, perf_mode=DR)
            pi, slot = mi // 2, mi % 2
            key = (sj, pi)
            if key not in er_tiles:
                if snw > SEG:
                    er_tiles[key] = e8p.tile([P, 2, 2, SEG], F8, tag="er",
                                             name=f"er{sj}_{pi}")
                else:
                    er_tiles[key] = e8s.tile([P, 2, 1, SEG], F8, tag="ers",
                                             name=f"er{sj}_{pi}")
            er = er_tiles[key]
            if snw > SEG:
                nc.scalar.activation(er[:mw, slot, :, :], sp[:mw, :, :], AF.Exp,
                                     scale=rd[:mw, mi : mi + 1])
            else:
                nc.scalar.activation(er[:mw, slot, 0, :snw], sp[:mw, 0, :snw],
                                     AF.Exp, scale=rd[:mw, mi : mi + 1])

        av_mid_q = []
        av_back_q = []

        def av_flush_back():
            while av_mid_q:
                av_mid_q.pop(0)()
            while av_back_q:
                av_back_q.pop(0)()

        def av_out_sub(sj, s):
                sn0, snw = SUPERS[sj]
                bw = min(P, snw - s * P)
                hh, c0 = s // 4, (s % 4) * P
                acc = aux.tile([P, SEG], F32, tag="aux", name=f"acc{sj}_{s}")
                for pi in range(12):
                    er = er_tiles[(sj, pi)]
                    nc.tensor.matmul(acc[:bw, : C + 2],
                                     er[:, :, hh, c0 : c0 + bw],
                                     pvT8[:, pi, :, :],
                                     start=(pi == 0), stop=False, perf_mode=DR)
                er = er_tiles[(sj, 12)]
                lmw = M_CHUNKS[24][1]
                nc.tensor.matmul(acc[:bw, : C + 2],
                                 er[:lmw, 0, hh, c0 : c0 + bw],
                                 pvT8[:lmw, 12, 0, :],
                                 start=False, stop=True)
                rc = smls.tile([P, 1], F32, tag="rc", name=f"rc{sj}_{s}")
                nc.vector.reciprocal(rc[:bw], acc[:bw, C : C + 1])
                un = scrp.tile([P, C], BF16, tag="un", name=f"un{sj}_{s}")
                nc.vector.tensor_scalar(un[:bw], acc[:bw, :C], rc[:bw], None,
                                        ALU.mult)
                pos = sn0 + s * P

                def mid(un=un, bw=bw, sj=sj, s=s, pos=pos):
                    # both c-chunks transpose into ONE psum tile (2nd matmul
                    # start=False accumulates into the already-zeroed region)
                    tp = aux.tile([P, 2, SEG], BF16, tag="aux",
                                  name=f"tp{sj}_{s}")
                    for cb in range(CC):
                        nc.tensor.matmul(tp[:, cb, :bw],
                                         un[:bw, cb * P : (cb + 1) * P],
                                         ident[:bw, :bw], is_transpose=True,
                                         start=(cb == 0), stop=(cb == CC - 1))

                    def back():
                        ot = outp.tile([P, CC, P], BF16, tag="ot",
                                       name=f"ot{sj}_{s}")
                        nc.vector.scalar_tensor_tensor(
                            ot[:, :, :bw], tp[:, :, :bw], 1.0 / WSCALE,
                            xq_f[:, :, pos : pos + bw], ALU.mult, ALU.add)
                        if use_bias:
                            for cb in range(CC):
                                nc.vector.tensor_scalar(ot[:, cb, :bw],
                                                        ot[:, cb, :bw],
                                                        bo_sb[:, cb : cb + 1],
                                                        None, ALU.add)
                        nc.sync.dma_start(out_v[:, :, pos : pos + bw],
                                          ot[:, :, :bw])

                    av_back_q.append(back)

                # stage the PE transposes one sub behind the AV matmuls and
                # the DVE output STT two behind, so neither engine's FIFO
                # ever stalls on a cross-engine round-trip
                av_mid_q.append(mid)
                if len(av_mid_q) > 1:
                    av_mid_q.pop(0)()
                if len(av_back_q) > 1:
                    av_back_q.pop(0)()

        def av_out_super(sj):
            snw = SUPERS[sj][1]
            for s in range((snw + P - 1) // P):
                av_out_sub(sj, s)

        # Work-queue emission: an exp for (super sj, chunk mi) is ready once
        # the q8 segs covering the super and the k8/rd seg covering the chunk
        # are computed. Emitting in availability order keeps the ACT queue
        # full from ~seg 2 onward. AV/output subtiles of completed supers are
        # interleaved between exps so the PE queue always has ready work.
        sup_ready_at = [(sn0 + snw - 1) // SEG for sn0, snw in SUPERS]
        n_chunks = len(M_CHUNKS)
        done_chunks = [set() for _ in SUPERS]
        av_pending = []
        av_done = 0
        FILL = 3

        def emit_av(k):
            nonlocal av_done
            while av_done < k and av_done < len(av_pending):
                av_out_sub(*av_pending[av_done])
                av_done += 1

        def emit_exp(sj, mi, av_rate=1):
            if mi in done_chunks[sj]:
                return
            s_exp_chunk(sj, mi)
            done_chunks[sj].add(mi)
            if len(done_chunks[sj]) == n_chunks:
                nsub = (SUPERS[sj][1] + P - 1) // P
                av_pending.extend((sj, s) for s in range(nsub))
            emit_av(av_done + av_rate)

        dma_seg(0)
        preamble()
        for si in range(len(SEGS)):
            if si + 1 < len(SEGS):
                dma_seg(si + 1)
            n0, nw = SEGS[si]
            norms_seg(si)
            proj_for_seg(n0, nw, on_act=(si <= 1))
            for mi2 in PV_PLAN.get(si, ()):
                pv_for_chunk(mi2, *M_CHUNKS[mi2])
            avail = min(4 * (si + 1), n_chunks)
            # Once the last seg lands, the final chunk's exp gates EVERY
            # super's AV: emit all supers' chunk 24 first so AV work can
            # start executing while the remaining exps drain.
            if avail == n_chunks:
                for sj in range(len(SUPERS)):
                    emit_exp(sj, n_chunks - 1)
            # super 0 has priority: finish earlier supers first and fill ACT
            # with just a little of the next super to avoid gaps.
            if sup_ready_at[0] <= si:
                for mi in range(avail):
                    emit_exp(0, mi)
            if si >= 1:
                for sj in range(1, len(SUPERS)):
                    if sup_ready_at[sj] > si or len(done_chunks[sj]) >= avail:
                        continue
                    take = 0
                    for mi in range(avail):
                        if take >= FILL:
                            break
                        if mi not in done_chunks[sj]:
                            emit_exp(sj, mi)
                            take += 1
                    break
        # drain remaining supers, AV interleaved; the tiny last super is
        # drained second-to-last so the final super's exps cover its AV
        # Drain order: super 1, most of super 2, super 3, then the held-back
        # tail of super 2. Super 3's AV (gated by its last exp) then overlaps
        # the held-back window, and super 2's AV overlaps its own tail exps.
        for sj in range(1, len(SUPERS)):
            for mi in range(n_chunks):
                emit_exp(sj, mi, av_rate=1)
        emit_av(len(av_pending))
        av_flush_back()

    return nc


_CACHE = {}


def _get_module(use_bias: bool):
    if use_bias not in _CACHE:
        nc = build(use_bias)
        nc.finalize()
        _CACHE[use_bias] = nc
    return _CACHE[use_bias]


def kernel(x_q, x_kv, Wq, bq, Wkv, bkv, Wproj, bproj):
    x_q = np.asarray(x_q, dtype=np.float32)
    x_kv = np.asarray(x_kv, dtype=np.float32)
    Wq = np.asarray(Wq, dtype=np.float32)
    bq = np.asarray(bq, dtype=np.float32)
    Wkv = np.asarray(Wkv, dtype=np.float32)
    bkv = np.asarray(bkv, dtype=np.float32)
    Wproj = np.asarray(Wproj, dtype=np.float32)
    bproj = np.asarray(bproj, dtype=np.float32)

    B, c, H, W = x_q.shape
    assert (c, H * W) == (C, N), (x_q.shape,)
    FP8 = ml_dtypes.float8_e4m3
    xq32 = x_q.reshape(B, C, N)
    xq = np.ascontiguousarray(xq32).astype(ml_dtypes.bfloat16)
    x8q = np.ascontiguousarray(xq32).astype(FP8)
    xkv8 = np.ascontiguousarray(x_kv.reshape(B, C, N)).astype(FP8)

    Wk = Wkv[:C]
    Wv = Wkv[C:]
    wq8 = np.ascontiguousarray(WSCALE * Wq.T).astype(FP8)
    wk8 = np.ascontiguousarray(WSCALE * Wk.T).astype(FP8)
    w38 = np.ascontiguousarray(WSCALE * (Wproj @ Wv).T).astype(FP8)
    gw8 = np.ascontiguousarray(WSCALE * (Wq.T @ Wk)).astype(FP8)
    bq16 = np.ascontiguousarray(WSCALE * bq)
    bk16 = np.ascontiguousarray(WSCALE * bkv[:C])
    bo = np.ascontiguousarray(Wproj @ bkv[C:] + bproj)

    use_bias = bool(np.any(bq16) or np.any(bk16) or np.any(bo))
    nc = _get_module(use_bias)

    in_maps = [
        {
            "xq": xq[b],
            "x8q": x8q[b],
            "xkv8": xkv8[b],
            "wq8": wq8,
            "wk8": wk8,
            "w38": w38,
            "gw8": gw8,
            "bq16": bq16,
            "bk16": bk16,
            "bo": bo,
        }
        for b in range(B)
    ]
    res = run_bass_kernel_spmd(nc, in_maps, core_ids=list(range(B)))
    out = np.stack([np.asarray(res.results[b]["out"]).astype(np.float32)
                    for b in range(B)], axis=0)
    return out.reshape(B, C, H, W)


# revision 6
# speedup vs baseline: 1.0065x; 1.0065x over previous
"""CrossAttention Trainium2 kernel, fp8 DoubleRow edition (1 batch elem/core).

Math per batch element (C=256 channels, N=3136 positions):
    q = Wq xq, k = Wk xkv, pv = (Wproj Wv) xkv          (1x1 convs)
    S[n,m] = q[:,n].k[:,m];  rd[m] = 1/(||q[:,m]|| ||k[:,m]|| + eps)
    A = softmax(S * rd[m] along m);  out = Wproj(A v) + xq residual

Implementation notes:
  * All matmuls in fp8e4 DoubleRow mode (2 fp8 rows/cycle, K=256 in one op).
    Weights and x are scaled by 16 on the host so w entries ~N(0,1) stay out
    of fp8 subnormals. Scales self-cancel in softmax: q,k scale 16 each =>
    S x256, qn2*kn2 x65536, rd = rsqrt(qn2*kn2) absorbs it. pv scale 16 is
    divided out in the output STT.
  * S^T tiles [m=128, n<=1024] (2 psum banks) -> ONE ACT Exp per (chunk,
    n-super) with per-partition scale rd[:,mi], writing fp8 er directly.
    |S*rd| <= ~1.1 so no max-subtraction is needed (exp in [0.3, 3.1]).
  * Norms via transposed DR projections (qT [m,256] psum) + DVE bn_stats
    (sum(x^2) = n*(var+mean^2)); rd = rsqrt via bit-trick + 2 Newton steps
    split DVE/gpsimd (ACT Sqrt would thrash the Exp table; gpsimd cannot
    read PSUM; tensor_scalar is DVE-only).
  * AV: er (stationary) x pvT|1,1 (moving) accumulating [n-sub, C+2]; the
    ones channels give the softmax row-sum for free. Normalize on DVE,
    bf16 PE transpose back to [c, n], fused (tp/16 + xq) STT, DMA out.
  * PSUM: S pool 2x2 banks + aux pool 4x1 bank (phase-1 qT/kT/pv/proj tiles,
    then AV acc + transpose) = 8 banks exactly.
"""

import sys

if "/opt/trn_rl_repo" not in sys.path:
    sys.path.insert(0, "/opt/trn_rl_repo")

import numpy as np
import ml_dtypes

import concourse.bass as bass
import concourse.mybir as mybir
import concourse.tile as tile
from concourse import bacc
from concourse.bass_utils import run_bass_kernel_spmd
from concourse.masks import make_identity
from contextlib import ExitStack

F32 = mybir.dt.float32
F8 = mybir.dt.float8e4
BF16 = mybir.dt.bfloat16
I32 = mybir.dt.int32
AF = mybir.ActivationFunctionType
DR = mybir.MatmulPerfMode.DoubleRow
ALU = mybir.AluOpType

P = 128
C = 256
CC = C // P            # 2
N = 56 * 56            # 3136
SEG = 512
SEGS = [(i, min(SEG, N - i)) for i in range(0, N, SEG)]            # 7
M_CHUNKS = [(i, min(P, N - i)) for i in range(0, N, P)]            # 25
N_FULL = len(M_CHUNKS) - 1                                         # 24 full
SUPERS = [(0, 1024), (1024, 1024), (2048, 1024), (3072, 64)]
WSCALE = 16.0          # host-side weight/x scale for fp8 range
# pv is needed only once AV starts (after the last seg): keep its psum
# traffic and DVE casts out of the phase-1 production pipeline entirely
PV_PLAN = {6: range(0, 25)}


def _chunks_of_seg(si):
    lo = si * 4
    return [(mi, M_CHUNKS[mi][0], M_CHUNKS[mi][1])
            for mi in range(lo, min(lo + 4, len(M_CHUNKS)))]


def build(use_bias: bool):
    nc = bacc.Bacc(None, target_bir_lowering=False)

    xq_d = nc.dram_tensor("xq", [C, N], BF16, kind="ExternalInput")
    x8q_d = nc.dram_tensor("x8q", [C, N], F8, kind="ExternalInput")
    xkv8_d = nc.dram_tensor("xkv8", [C, N], F8, kind="ExternalInput")
    wq8_d = nc.dram_tensor("wq8", [C, C], F8, kind="ExternalInput")   # [c, d]
    wk8_d = nc.dram_tensor("wk8", [C, C], F8, kind="ExternalInput")
    w38_d = nc.dram_tensor("w38", [C, C], F8, kind="ExternalInput")
    gw8_d = nc.dram_tensor("gw8", [C, C], F8, kind="ExternalInput")   # 16*Wq.T@Wk
    bq_d = nc.dram_tensor("bq16", [C], F32, kind="ExternalInput")     # 16*bq
    bk_d = nc.dram_tensor("bk16", [C], F32, kind="ExternalInput")     # 16*bk
    bo_d = nc.dram_tensor("bo", [C], F32, kind="ExternalInput")       # Wproj bv + bproj
    out_d = nc.dram_tensor("out", [C, N], BF16, kind="ExternalOutput")

    xq_v = xq_d[:].rearrange("(cc p) n -> p cc n", p=P)
    x8q_v = x8q_d[:].rearrange("(cc p) n -> p cc n", p=P)
    xkv8_v = xkv8_d[:].rearrange("(cc p) n -> p cc n", p=P)
    out_v = out_d[:].rearrange("(cc p) n -> p cc n", p=P)

    with tile.TileContext(nc) as tc, ExitStack() as ctx:
        pers = ctx.enter_context(tc.tile_pool(name="pers", bufs=1))
        sS = ctx.enter_context(tc.tile_pool(name="sS", bufs=2, space="PSUM"))
        aux = ctx.enter_context(tc.tile_pool(name="aux", bufs=4, space="PSUM"))
        scrp = ctx.enter_context(tc.tile_pool(name="scrp", bufs=3))
        e8p = ctx.enter_context(tc.tile_pool(name="e8p", bufs=39))
        e8s = ctx.enter_context(tc.tile_pool(name="e8s", bufs=13))
        outp = ctx.enter_context(tc.tile_pool(name="outp", bufs=4))
        smls = ctx.enter_context(tc.tile_pool(name="smls", bufs=4))

        xq_f = pers.tile([P, CC, N], BF16)
        x8q = pers.tile([P, CC, N], F8)
        x8kv = pers.tile([P, CC, N], F8)
        if use_bias:
            q8 = pers.tile([P, CC, N], F8)
            k8 = pers.tile([P, CC, N], F8)
        else:
            # S = xkv^T (Wk^T Wq) xq: one fused projection g = G xq replaces
            # both q and k projections for the S matmul (norms still use
            # the transposed qT/kT products)
            g8 = pers.tile([P, CC, N], F8)
            gw8 = pers.tile([P, CC, C], F8)
        wq8 = pers.tile([P, CC, C], F8)
        wk8 = pers.tile([P, CC, C], F8)
        w38 = pers.tile([P, CC, C], F8)
        pvT8 = pers.tile([P, 13, 2, C + 2], F8)
        rd = pers.tile([P, len(M_CHUNKS)], F32)
        if use_bias:
            qn2 = pers.tile([P, len(M_CHUNKS)], F32)
            kn2 = pers.tile([P, len(M_CHUNKS)], F32)
        ident = pers.tile([P, P], BF16)
        if use_bias:
            bq_sb = pers.tile([P, CC], F32)
            bk_sb = pers.tile([P, CC], F32)
            bo_sb = pers.tile([P, CC], F32)
            bqb = pers.tile([P, C], F32)
            bkb = pers.tile([P, C], F32)

        def preamble():
            make_identity(nc, ident)
            nc.sync.dma_start(wq8, wq8_d[:].rearrange("(cc p) d -> p cc d", p=P))
            nc.sync.dma_start(wk8, wk8_d[:].rearrange("(cc p) d -> p cc d", p=P))
            nc.sync.dma_start(w38, w38_d[:].rearrange("(cc p) d -> p cc d", p=P))
            if not use_bias:
                nc.sync.dma_start(gw8,
                                  gw8_d[:].rearrange("(cc p) d -> p cc d", p=P))
            # softmax row-sum channels
            nc.vector.memset(pvT8[:, :, :, C : C + 2], 1.0)
        if use_bias:
            nc.sync.dma_start(bq_sb, bq_d[:].rearrange("(c p) -> p c", p=P))
            nc.sync.dma_start(bk_sb, bk_d[:].rearrange("(c p) -> p c", p=P))
            nc.sync.dma_start(bo_sb, bo_d[:].rearrange("(c p) -> p c", p=P))
            nc.sync.dma_start(
                bqb, bass.AP(tensor=bq_d[:].tensor, offset=0, ap=[[0, P], [1, C]])
            )
            nc.sync.dma_start(
                bkb, bass.AP(tensor=bk_d[:].tensor, offset=0, ap=[[0, P], [1, C]])
            )

        # ---------------- phase 1 (per 512-seg, pipelined) ----------------
        def norms_for_chunk(mi, m0, mw, ci, bag):
            for side, (which, xsrc, w8) in enumerate(
                (("q", x8q, wq8), ("k", x8kv, wk8))
            ):
                ps = aux.tile([P, SEG], F32, tag="aux", name=f"t{which}{m0}")
                nc.tensor.matmul(ps[:mw, :C], xsrc[:, :, m0 : m0 + mw], w8,
                                 start=True, stop=True, perf_mode=DR)
                if use_bias:
                    nacc = qn2 if which == "q" else kn2
                    bbt = bqb if which == "q" else bkb
                    scr = scrp.tile([P, C], F32, tag="sq", name=f"s{which}{m0}")
                    nc.vector.tensor_add(scr[:mw], ps[:mw, :C], bbt[:mw])
                    nc.vector.scalar_tensor_tensor(
                        scr[:mw], scr[:mw], 1.0, scr[:mw], ALU.mult, ALU.mult,
                        accum_out=nacc[:mw, mi : mi + 1])
                else:
                    # sum(x^2) = n*(var + mean^2) via bn_stats: one DVE pass
                    # over the PSUM tile instead of copy+square+reduce
                    bn6 = scrp.tile([P, 2, 6], F32, tag="bn6",
                                    name=f"b{which}{m0}")
                    nc.vector.bn_stats(bn6[:mw, side, :], ps[:mw, :C])
                    nc.vector.bn_aggr(bag[:mw, ci, side, :], bn6[:mw, side, :])

        def pv_for_chunk(mi, m0, mw):
            ps = aux.tile([P, SEG], F32, tag="aux", name=f"pv{m0}")
            nc.tensor.matmul(ps[:mw, :C], x8kv[:, :, m0 : m0 + mw], w38,
                             start=True, stop=True, perf_mode=DR)
            nc.vector.tensor_copy(pvT8[:mw, mi // 2, mi % 2, :C], ps[:mw, :C])

        def rd_for_seg(si, bag, part=None):
            lo = si * 4
            hi = min(lo + 4, len(M_CHUNKS))
            nseg = hi - lo
            c0_, c1_ = part if part else (0, nseg)
            u = smls.tile([P, 4], F32, tag="u", name=f"u{si}_{c0_}")
            sl = slice(c0_, c1_)
            if use_bias:
                nc.gpsimd.tensor_mul(u[:, sl], qn2[:, lo + c0_ : lo + c1_],
                                     kn2[:, lo + c0_ : lo + c1_])
            else:
                t = smls.tile([P, 4, 2, 1], F32, tag="tvm",
                              name=f"tvm{si}_{c0_}")
                mean = bag[:, sl, :, 0:1]
                var = bag[:, sl, :, 1:2]
                nc.gpsimd.tensor_mul(t[:, sl], mean, mean)
                nc.gpsimd.tensor_add(t[:, sl], t[:, sl], var)
                nc.gpsimd.tensor_mul(u[:, sl], t[:, sl, 0, 0],
                                     t[:, sl, 1, 0])
            yb = smls.tile([P, 4], I32, tag="yb", name=f"yb{si}_{c0_}")
            nc.vector.tensor_scalar(yb[:, sl], u[:, sl].bitcast(I32),
                                    1, None, ALU.logical_shift_right)
            nc.vector.tensor_scalar(yb[:, sl], yb[:, sl], -1, 0x5F3759DF,
                                    ALU.mult, ALU.add)
            y = yb.bitcast(F32)
            h = smls.tile([P, 4], F32, tag="h", name=f"h{si}_{c0_}")
            for _ in range(2):
                nc.gpsimd.tensor_mul(h[:, sl], y[:, sl], y[:, sl])
                nc.gpsimd.tensor_mul(h[:, sl], h[:, sl], u[:, sl])
                nc.vector.tensor_scalar(h[:, sl], h[:, sl], -0.5, 1.5,
                                        ALU.mult, ALU.add)
                nc.gpsimd.tensor_mul(y[:, sl], y[:, sl], h[:, sl])
            if use_bias:
                nc.gpsimd.tensor_copy(rd[:, lo + c0_ : lo + c1_], y[:, sl])
            else:
                # g-fold S is 16*S; u = (qn*kn/256)^2 => rd = rsqrt(u)/16
                nc.vector.tensor_scalar(rd[:, lo + c0_ : lo + c1_], y[:, sl],
                                        1.0 / 16.0, None, ALU.mult)

        def proj_for_seg(n0, nw, on_act=False):
            if use_bias:
                plan = (("q", x8q, wq8, q8, bq_sb), ("k", x8kv, wk8, k8, bk_sb))
            else:
                plan = (("g", x8q, gw8, g8, None),)
            for which, xsrc, w8, dst, bt in plan:
                for dc in range(CC):
                    ps = aux.tile([P, SEG], F32, tag="aux",
                                  name=f"p{which}{n0}_{dc}")
                    nc.tensor.matmul(ps[:, :nw], w8[:, :, dc * P : (dc + 1) * P],
                                     xsrc[:, :, n0 : n0 + nw],
                                     start=True, stop=True, perf_mode=DR)
                    if use_bias:
                        nc.vector.tensor_scalar(dst[:, dc, n0 : n0 + nw],
                                                ps[:, :nw], bt[:, dc : dc + 1],
                                                None, ALU.add)
                    elif on_act:
                        # lead-in: ACT Copy shares the Exp table and shortens
                        # the DVE dep chain in front of the first exps
                        nc.scalar.activation(dst[:, dc, n0 : n0 + nw],
                                             ps[:, :nw], AF.Copy)
                    else:
                        nc.vector.tensor_copy(dst[:, dc, n0 : n0 + nw],
                                              ps[:, :nw])

        def dma_seg(si):
            n0, nw = SEGS[si]
            nc.sync.dma_start(x8q[:, :, n0 : n0 + nw],
                              x8q_v[:, :, n0 : n0 + nw])
            nc.sync.dma_start(x8kv[:, :, n0 : n0 + nw],
                              xkv8_v[:, :, n0 : n0 + nw])
            nc.sync.dma_start(xq_f[:, :, n0 : n0 + nw],
                              xq_v[:, :, n0 : n0 + nw])

        def norms_seg(si):
            bag = smls.tile([P, 4, 2, 2], F32, tag="bag", name=f"bag{si}")
            for ci, (mi, m0, mw) in enumerate(_chunks_of_seg(si)):
                norms_for_chunk(mi, m0, mw, ci, bag)
                if si == 0 and ci == 0:
                    # chunk 0's rd alone unblocks the very first exp
                    rd_for_seg(0, bag, part=(0, 1))
            if si == 0:
                rd_for_seg(si, bag, part=(1, 4))
            else:
                rd_for_seg(si, bag)

        # ---------------- phase 2 ----------------
        er_tiles = {}

        def s_exp_chunk(sj, mi):
            sn0, snw = SUPERS[sj]
            m0, mw = M_CHUNKS[mi]
            sp = sS.tile([P, 2, SEG], F32, tag="sp", name=f"sp{sj}_{mi}")
            lhsT = k8 if use_bias else x8kv
            rhs = q8 if use_bias else g8
            nh = (snw + SEG - 1) // SEG
            for hi in range(nh):
                hw = min(SEG, snw - hi * SEG)
                nc.tensor.matmul(sp[:mw, hi, :hw], lhsT[:, :, m0 : m0 + mw],
                                 rhs[:, :, sn0 + hi * SEG : sn0 + hi * SEG + hw],
                                 start=True, stop=True, perf_mode=DR)
            pi, slot = mi // 2, mi % 2
            key = (sj, pi)
            if key not in er_tiles:
                if snw > SEG:
                    er_tiles[key] = e8p.tile([P, 2, 2, SEG], F8, tag="er",
                                             name=f"er{sj}_{pi}")
                else:
                    er_tiles[key] = e8s.tile([P, 2, 1, SEG], F8, tag="ers",
                                             name=f"er{sj}_{pi}")
            er = er_tiles[key]
            if snw > SEG:
                nc.scalar.activation(er[:mw, slot, :, :], sp[:mw, :, :], AF.Exp,
                                     scale=rd[:mw, mi : mi + 1])
            else:
                nc.scalar.activation(er[:mw, slot, 0, :snw], sp[:mw, 0, :snw],
                                     AF.Exp, scale=rd[:mw, mi : mi + 1])

        av_mid_q = []
        av_back_q = []

        def av_flush_back():
            while av_mid_q:
                av_mid_q.pop(0)()
            while av_back_q:
                av_back_q.pop(0)()

        def av_out_sub(sj, s):
                sn0, snw = SUPERS[sj]
                bw = min(P, snw - s * P)
                hh, c0 = s // 4, (s % 4) * P
                acc = aux.tile([P, SEG], F32, tag="aux", name=f"acc{sj}_{s}")
                for pi in range(12):
                    er = er_tiles[(sj, pi)]
                    nc.tensor.matmul(acc[:bw, : C + 2],
                                     er[:, :, hh, c0 : c0 + bw],
                                     pvT8[:, pi, :, :],
                                     start=(pi == 0), stop=False, perf_mode=DR)
                er = er_tiles[(sj, 12)]
                lmw = M_CHUNKS[24][1]
                nc.tensor.matmul(acc[:bw, : C + 2],
                                 er[:lmw, 0, hh, c0 : c0 + bw],
                                 pvT8[:lmw, 12, 0, :],
                                 start=False, stop=True)
                rc = smls.tile([P, 1], F32, tag="rc", name=f"rc{sj}_{s}")
                nc.vector.reciprocal(rc[:bw], acc[:bw, C : C + 1])
                un = scrp.tile([P, C], BF16, tag="un", name=f"un{sj}_{s}")
                nc.vector.tensor_scalar(un[:bw], acc[:bw, :C], rc[:bw], None,
                                        ALU.mult)
                pos = sn0 + s * P

                def mid(un=un, bw=bw, sj=sj, s=s, pos=pos):
                    # both c-chunks transpose into ONE psum tile (2nd matmul
                    # start=False accumulates into the already-zeroed region)
                    tp = aux.tile([P, 2, SEG], BF16, tag="aux",
                                  name=f"tp{sj}_{s}")
                    for cb in range(CC):
                        nc.tensor.matmul(tp[:, cb, :bw],
                                         un[:bw, cb * P : (cb + 1) * P],
                                         ident[:bw, :bw], is_transpose=True,
                                         start=(cb == 0), stop=(cb == CC - 1))

                    def back():
                        ot = outp.tile([P, CC, P], BF16, tag="ot",
                                       name=f"ot{sj}_{s}")
                        nc.vector.scalar_tensor_tensor(
                            ot[:, :, :bw], tp[:, :, :bw], 1.0 / WSCALE,
                            xq_f[:, :, pos : pos + bw], ALU.mult, ALU.add)
                        if use_bias:
                            for cb in range(CC):
                                nc.vector.tensor_scalar(ot[:, cb, :bw],
                                                        ot[:, cb, :bw],
                                                        bo_sb[:, cb : cb + 1],
                                                        None, ALU.add)
                        nc.sync.dma_start(out_v[:, :, pos : pos + bw],
                                          ot[:, :, :bw])

                    av_back_q.append(back)

                # stage the PE transposes one sub behind the AV matmuls and
                # the DVE output STT two behind, so neither engine's FIFO
                # ever stalls on a cross-engine round-trip
                av_mid_q.append(mid)
                if len(av_mid_q) > 1:
                    av_mid_q.pop(0)()
                if len(av_back_q) > 1:
                    av_back_q.pop(0)()

        def av_out_super(sj):
            snw = SUPERS[sj][1]
            for s in range((snw + P - 1) // P):
                av_out_sub(sj, s)

        # Work-queue emission: an exp for (super sj, chunk mi) is ready once
        # the q8 segs covering the super and the k8/rd seg covering the chunk
        # are computed. Emitting in availability order keeps the ACT queue
        # full from ~seg 2 onward. AV/output subtiles of completed supers are
        # interleaved between exps so the PE queue always has ready work.
        sup_ready_at = [(sn0 + snw - 1) // SEG for sn0, snw in SUPERS]
        n_chunks = len(M_CHUNKS)
        done_chunks = [set() for _ in SUPERS]
        av_pending = []
        av_done = 0
        FILL = 3

        def emit_av(k):
            nonlocal av_done
            while av_done < k and av_done < len(av_pending):
                av_out_sub(*av_pending[av_done])
                av_done += 1

        def emit_exp(sj, mi, av_rate=1):
            if mi in done_chunks[sj]:
                return
            s_exp_chunk(sj, mi)
            done_chunks[sj].add(mi)
            if len(done_chunks[sj]) == n_chunks:
                nsub = (SUPERS[sj][1] + P - 1) // P
                av_pending.extend((sj, s) for s in range(nsub))
            emit_av(av_done + av_rate)

        dma_seg(0)
        preamble()
        for si in range(len(SEGS)):
            if si + 1 < len(SEGS):
                dma_seg(si + 1)
            n0, nw = SEGS[si]
            norms_seg(si)
            proj_for_seg(n0, nw, on_act=(si <= 1))
            for mi2 in PV_PLAN.get(si, ()):
                pv_for_chunk(mi2, *M_CHUNKS[mi2])
            avail = min(4 * (si + 1), n_chunks)
            # Once the last seg lands, the final chunk's exp gates EVERY
            # super's AV: emit all supers' chunk 24 first so AV work can
            # start executing while the remaining exps drain.
            if avail == n_chunks:
                for sj in range(len(SUPERS)):
                    emit_exp(sj, n_chunks - 1)
            # super 0 has priority: finish earlier supers first and fill ACT
            # with just a little of the next super to avoid gaps.
            if sup_ready_at[0] <= si:
                for mi in range(avail):
                    emit_exp(0, mi)
            if si >= 1:
                for sj in range(1, len(SUPERS)):
                    if sup_ready_at[sj] > si or len(done_chunks[sj]) >= avail:
                        continue
                    take = 0
                    for mi in range(avail):
                        if take >= FILL:
                            break
                        if mi not in done_chunks[sj]:
                            emit_exp(sj, mi)
                            take += 1
                    break
        # drain remaining supers, AV interleaved; the tiny last super is
        # drained second-to-last so the final super's exps cover its AV
        # Drain order: super 1, most of super 2, super 3, then the held-back
        # tail of super 2. Super 3's AV (gated by its last exp) then overlaps
        # the held-back window, and super 2's AV overlaps its own tail exps.
        for sj in range(1, len(SUPERS)):
            for mi in range(n_chunks):
                emit_exp(sj, mi, av_rate=1)
        emit_av(len(av_pending))
        av_flush_back()

    return nc


_CACHE = {}


def _get_module(use_bias: bool):
    if use_bias not in _CACHE:
        nc = build(use_bias)
        nc.finalize()
        _CACHE[use_bias] = nc
    return _CACHE[use_bias]


def kernel(x_q, x_kv, Wq, bq, Wkv, bkv, Wproj, bproj):
    x_q = np.asarray(x_q, dtype=np.float32)
    x_kv = np.asarray(x_kv, dtype=np.float32)
    Wq = np.asarray(Wq, dtype=np.float32)
    bq = np.asarray(bq, dtype=np.float32)
    Wkv = np.asarray(Wkv, dtype=np.float32)
    bkv = np.asarray(bkv, dtype=np.float32)
    Wproj = np.asarray(Wproj, dtype=np.float32)
    bproj = np.asarray(bproj, dtype=np.float32)

    B, c, H, W = x_q.shape
    assert (c, H * W) == (C, N), (x_q.shape,)
    FP8 = ml_dtypes.float8_e4m3
    xq32 = x_q.reshape(B, C, N)
    xq = np.ascontiguousarray(xq32).astype(ml_dtypes.bfloat16)
    x8q = np.ascontiguousarray(xq32).astype(FP8)
    xkv8 = np.ascontiguousarray(x_kv.reshape(B, C, N)).astype(FP8)

    Wk = Wkv[:C]
    Wv = Wkv[C:]
    wq8 = np.ascontiguousarray(WSCALE * Wq.T).astype(FP8)
    wk8 = np.ascontiguousarray(WSCALE * Wk.T).astype(FP8)
    w38 = np.ascontiguousarray(WSCALE * (Wproj @ Wv).T).astype(FP8)
    gw8 = np.ascontiguousarray(WSCALE * (Wq.T @ Wk)).astype(FP8)
    bq16 = np.ascontiguousarray(WSCALE * bq)
    bk16 = np.ascontiguousarray(WSCALE * bkv[:C])
    bo = np.ascontiguousarray(Wproj @ bkv[C:] + bproj)

    use_bias = bool(np.any(bq16) or np.any(bk16) or np.any(bo))
    nc = _get_module(use_bias)

    in_maps = [
        {
            "xq": xq[b],
            "x8q": x8q[b],
            "xkv8": xkv8[b],
            "wq8": wq8,
            "wk8": wk8,
            "w38": w38,
            "gw8": gw8,
            "bq16": bq16,
            "bk16": bk16,
            "bo": bo,
        }
        for b in range(B)
    ]
    res = run_bass_kernel_spmd(nc, in_maps, core_ids=list(range(B)))
    out = np.stack([np.asarray(res.results[b]["out"]).astype(np.float32)
                    for b in range(B)], axis=0)
    return out.reshape(B, C, H, W)


# revision 8
# speedup vs baseline: 1.0338x; 1.0272x over previous
"""CrossAttention Trainium2 kernel, fp8 DoubleRow edition (1 batch elem/core).

Math per batch element (C=256 channels, N=3136 positions):
    q = Wq xq, k = Wk xkv, pv = (Wproj Wv) xkv          (1x1 convs)
    S[n,m] = q[:,n].k[:,m];  rd[m] = 1/(||q[:,m]|| ||k[:,m]|| + eps)
    A = softmax(S * rd[m] along m);  out = Wproj(A v) + xq residual

Implementation notes:
  * All matmuls in fp8e4 DoubleRow mode (2 fp8 rows/cycle, K=256 in one op).
    Weights and x are scaled by 16 on the host so w entries ~N(0,1) stay out
    of fp8 subnormals. Scales self-cancel in softmax: q,k scale 16 each =>
    S x256, qn2*kn2 x65536, rd = rsqrt(qn2*kn2) absorbs it. pv scale 16 is
    divided out in the output STT.
  * S^T tiles [m=128, n<=1024] (2 psum banks) -> ONE ACT Exp per (chunk,
    n-super) with per-partition scale rd[:,mi], writing fp8 er directly.
    |S*rd| <= ~1.1 so no max-subtraction is needed (exp in [0.3, 3.1]).
  * Norms via transposed DR projections (qT [m,256] psum) + DVE square +
    X-reduce; rd = rsqrt via DVE bit-trick + 2 Newton steps (ACT Sqrt would
    thrash the Exp activation table; gpsimd cannot read PSUM).
  * AV: er (stationary) x pvT|1,1 (moving) accumulating [n-sub, C+2]; the
    ones channels give the softmax row-sum for free. Normalize on DVE,
    bf16 PE transpose back to [c, n], fused (tp/16 + xq) STT, DMA out.
  * PSUM: S pool 2x2 banks + aux pool 4x1 bank (phase-1 qT/kT/pv/proj tiles,
    then AV acc + transpose) = 8 banks exactly.
"""

import sys

if "/opt/trn_rl_repo" not in sys.path:
    sys.path.insert(0, "/opt/trn_rl_repo")

import numpy as np
import ml_dtypes

import concourse.bass as bass
import concourse.mybir as mybir
import concourse.tile as tile
from concourse import bacc
from concourse.bass_utils import run_bass_kernel_spmd
from concourse.masks import make_identity
from contextlib import ExitStack

F32 = mybir.dt.float32
F8 = mybir.dt.float8e4
BF16 = mybir.dt.bfloat16
I32 = mybir.dt.int32
AF = mybir.ActivationFunctionType
DR = mybir.MatmulPerfMode.DoubleRow
ALU = mybir.AluOpType

P = 128
C = 256
CC = C // P            # 2
N = 56 * 56            # 3136
SEG = 512
SEGS = [(i, min(SEG, N - i)) for i in range(0, N, SEG)]            # 7
M_CHUNKS = [(i, min(P, N - i)) for i in range(0, N, P)]            # 25
N_FULL = len(M_CHUNKS) - 1                                         # 24 full
SUPERS = [(0, 1024), (1024, 1024), (2048, 1024), (3072, 64)]
WSCALE = 16.0          # host-side weight/x scale for fp8 range
# pv is needed only once AV starts (after the last seg): keep its psum
# traffic and DVE casts out of the phase-1 production pipeline entirely
PV_PLAN = {6: range(0, 25)}


def _chunks_of_seg(si):
    lo = si * 4
    return [(mi, M_CHUNKS[mi][0], M_CHUNKS[mi][1])
            for mi in range(lo, min(lo + 4, len(M_CHUNKS)))]


def build(use_bias: bool):
    nc = bacc.Bacc(None, target_bir_lowering=False)

    xq_d = nc.dram_tensor("xq", [C, N], BF16, kind="ExternalInput")
    x8q_d = nc.dram_tensor("x8q", [C, N], F8, kind="ExternalInput")
    xkv8_d = nc.dram_tensor("xkv8", [C, N], F8, kind="ExternalInput")
    wq8_d = nc.dram_tensor("wq8", [C, C], F8, kind="ExternalInput")   # [c, d]
    wk8_d = nc.dram_tensor("wk8", [C, C], F8, kind="ExternalInput")
    w38_d = nc.dram_tensor("w38", [C, C], F8, kind="ExternalInput")
    gw8_d = nc.dram_tensor("gw8", [C, C], F8, kind="ExternalInput")   # 16*Wq.T@Wk
    bq_d = nc.dram_tensor("bq16", [C], F32, kind="ExternalInput")     # 16*bq
    bk_d = nc.dram_tensor("bk16", [C], F32, kind="ExternalInput")     # 16*bk
    bo_d = nc.dram_tensor("bo", [C], F32, kind="ExternalInput")       # Wproj bv + bproj
    out_d = nc.dram_tensor("out", [C, N], BF16, kind="ExternalOutput")

    xq_v = xq_d[:].rearrange("(cc p) n -> p cc n", p=P)
    x8q_v = x8q_d[:].rearrange("(cc p) n -> p cc n", p=P)
    xkv8_v = xkv8_d[:].rearrange("(cc p) n -> p cc n", p=P)
    out_v = out_d[:].rearrange("(cc p) n -> p cc n", p=P)

    with tile.TileContext(nc) as tc, ExitStack() as ctx:
        pers = ctx.enter_context(tc.tile_pool(name="pers", bufs=1))
        sS = ctx.enter_context(tc.tile_pool(name="sS", bufs=2, space="PSUM"))
        aux = ctx.enter_context(tc.tile_pool(name="aux", bufs=4, space="PSUM"))
        scrp = ctx.enter_context(tc.tile_pool(name="scrp", bufs=3))
        e8p = ctx.enter_context(tc.tile_pool(name="e8p", bufs=39))
        e8s = ctx.enter_context(tc.tile_pool(name="e8s", bufs=13))
        outp = ctx.enter_context(tc.tile_pool(name="outp", bufs=4))
        smls = ctx.enter_context(tc.tile_pool(name="smls", bufs=4))

        xq_f = pers.tile([P, CC, N], BF16)
        x8q = pers.tile([P, CC, N], F8)
        x8kv = pers.tile([P, CC, N], F8)
        if use_bias:
            q8 = pers.tile([P, CC, N], F8)
            k8 = pers.tile([P, CC, N], F8)
        else:
            # S = xkv^T (Wk^T Wq) xq: one fused projection g = G xq replaces
            # both q and k projections for the S matmul (norms still use
            # the transposed qT/kT products)
            g8 = pers.tile([P, CC, N], F8)
            gw8 = pers.tile([P, CC, C], F8)
        wq8 = pers.tile([P, CC, C], F8)
        wk8 = pers.tile([P, CC, C], F8)
        w38 = pers.tile([P, CC, C], F8)
        pvT8 = pers.tile([P, 13, 2, C + 2], F8)
        rd = pers.tile([P, len(M_CHUNKS)], F32)
        if use_bias:
            qn2 = pers.tile([P, len(M_CHUNKS)], F32)
            kn2 = pers.tile([P, len(M_CHUNKS)], F32)
        ident = pers.tile([P, P], BF16)
        if use_bias:
            bq_sb = pers.tile([P, CC], F32)
            bk_sb = pers.tile([P, CC], F32)
            bo_sb = pers.tile([P, CC], F32)
            bqb = pers.tile([P, C], F32)
            bkb = pers.tile([P, C], F32)

        def preamble():
            make_identity(nc, ident)
            nc.sync.dma_start(wq8, wq8_d[:].rearrange("(cc p) d -> p cc d", p=P))
            nc.sync.dma_start(wk8, wk8_d[:].rearrange("(cc p) d -> p cc d", p=P))
            if not use_bias:
                nc.sync.dma_start(gw8,
                                  gw8_d[:].rearrange("(cc p) d -> p cc d", p=P))
            # softmax row-sum channels
            nc.vector.memset(pvT8[:, :, :, C : C + 2], 1.0)

        def preamble_late():
            # w38 feeds pv, which only runs after the last seg
            nc.sync.dma_start(w38, w38_d[:].rearrange("(cc p) d -> p cc d", p=P))
        if use_bias:
            nc.sync.dma_start(bq_sb, bq_d[:].rearrange("(c p) -> p c", p=P))
            nc.sync.dma_start(bk_sb, bk_d[:].rearrange("(c p) -> p c", p=P))
            nc.sync.dma_start(bo_sb, bo_d[:].rearrange("(c p) -> p c", p=P))
            nc.sync.dma_start(
                bqb, bass.AP(tensor=bq_d[:].tensor, offset=0, ap=[[0, P], [1, C]])
            )
            nc.sync.dma_start(
                bkb, bass.AP(tensor=bk_d[:].tensor, offset=0, ap=[[0, P], [1, C]])
            )

        # ---------------- phase 1 (per 512-seg, pipelined) ----------------
        def norms_for_chunk(mi, m0, mw, ci, bag):
            for side, (which, xsrc, w8) in enumerate(
                (("q", x8q, wq8), ("k", x8kv, wk8))
            ):
                ps = aux.tile([P, SEG], F32, tag="aux", name=f"t{which}{m0}")
                nc.tensor.matmul(ps[:mw, :C], xsrc[:, :, m0 : m0 + mw], w8,
                                 start=True, stop=True, perf_mode=DR)
                if use_bias:
                    nacc = qn2 if which == "q" else kn2
                    bbt = bqb if which == "q" else bkb
                    scr = scrp.tile([P, C], F32, tag="sq", name=f"s{which}{m0}")
                    nc.vector.tensor_add(scr[:mw], ps[:mw, :C], bbt[:mw])
                    nc.vector.scalar_tensor_tensor(
                        scr[:mw], scr[:mw], 1.0, scr[:mw], ALU.mult, ALU.mult,
                        accum_out=nacc[:mw, mi : mi + 1])
                else:
                    # sum(x^2) = n*(var + mean^2) via bn_stats: one DVE pass
                    # over the PSUM tile instead of copy+square+reduce
                    bn6 = scrp.tile([P, 2, 6], F32, tag="bn6",
                                    name=f"b{which}{m0}")
                    nc.vector.bn_stats(bn6[:mw, side, :], ps[:mw, :C])
                    nc.vector.bn_aggr(bag[:mw, ci, side, :], bn6[:mw, side, :])

        def pv_for_chunk(mi, m0, mw):
            ps = aux.tile([P, SEG], F32, tag="aux", name=f"pv{m0}")
            nc.tensor.matmul(ps[:mw, :C], x8kv[:, :, m0 : m0 + mw], w38,
                             start=True, stop=True, perf_mode=DR)
            nc.vector.tensor_copy(pvT8[:mw, mi // 2, mi % 2, :C], ps[:mw, :C])

        def rd_for_seg(si, bag, part=None):
            lo = si * 4
            hi = min(lo + 4, len(M_CHUNKS))
            nseg = hi - lo
            c0_, c1_ = part if part else (0, nseg)
            u = smls.tile([P, 4], F32, tag="u", name=f"u{si}_{c0_}")
            sl = slice(c0_, c1_)
            if use_bias:
                nc.gpsimd.tensor_mul(u[:, sl], qn2[:, lo + c0_ : lo + c1_],
                                     kn2[:, lo + c0_ : lo + c1_])
            else:
                t = smls.tile([P, 4, 2, 1], F32, tag="tvm",
                              name=f"tvm{si}_{c0_}")
                mean = bag[:, sl, :, 0:1]
                var = bag[:, sl, :, 1:2]
                nc.gpsimd.tensor_mul(t[:, sl], mean, mean)
                nc.gpsimd.tensor_add(t[:, sl], t[:, sl], var)
                nc.gpsimd.tensor_mul(u[:, sl], t[:, sl, 0, 0],
                                     t[:, sl, 1, 0])
            yb = smls.tile([P, 4], I32, tag="yb", name=f"yb{si}_{c0_}")
            nc.vector.tensor_scalar(yb[:, sl], u[:, sl].bitcast(I32),
                                    1, None, ALU.logical_shift_right)
            nc.vector.tensor_scalar(yb[:, sl], yb[:, sl], -1, 0x5F3759DF,
                                    ALU.mult, ALU.add)
            y = yb.bitcast(F32)
            h = smls.tile([P, 4], F32, tag="h", name=f"h{si}_{c0_}")
            for _ in range(2):
                nc.gpsimd.tensor_mul(h[:, sl], y[:, sl], y[:, sl])
                nc.gpsimd.tensor_mul(h[:, sl], h[:, sl], u[:, sl])
                nc.vector.tensor_scalar(h[:, sl], h[:, sl], -0.5, 1.5,
                                        ALU.mult, ALU.add)
                nc.gpsimd.tensor_mul(y[:, sl], y[:, sl], h[:, sl])
            if use_bias:
                nc.gpsimd.tensor_copy(rd[:, lo + c0_ : lo + c1_], y[:, sl])
            else:
                # g-fold S is 16*S; u = (qn*kn/256)^2 => rd = rsqrt(u)/16
                nc.vector.tensor_scalar(rd[:, lo + c0_ : lo + c1_], y[:, sl],
                                        1.0 / 16.0, None, ALU.mult)

        def proj_for_seg(n0, nw, on_act=False):
            if use_bias:
                plan = (("q", x8q, wq8, q8, bq_sb), ("k", x8kv, wk8, k8, bk_sb))
            else:
                plan = (("g", x8q, gw8, g8, None),)
            for which, xsrc, w8, dst, bt in plan:
                for dc in range(CC):
                    ps = aux.tile([P, SEG], F32, tag="aux",
                                  name=f"p{which}{n0}_{dc}")
                    nc.tensor.matmul(ps[:, :nw], w8[:, :, dc * P : (dc + 1) * P],
                                     xsrc[:, :, n0 : n0 + nw],
                                     start=True, stop=True, perf_mode=DR)
                    if use_bias:
                        nc.vector.tensor_scalar(dst[:, dc, n0 : n0 + nw],
                                                ps[:, :nw], bt[:, dc : dc + 1],
                                                None, ALU.add)
                    elif on_act:
                        # lead-in: ACT Copy shares the Exp table and shortens
                        # the DVE dep chain in front of the first exps
                        nc.scalar.activation(dst[:, dc, n0 : n0 + nw],
                                             ps[:, :nw], AF.Copy)
                    else:
                        nc.vector.tensor_copy(dst[:, dc, n0 : n0 + nw],
                                              ps[:, :nw])

        def dma_seg(si):
            n0, nw = SEGS[si]
            nc.sync.dma_start(x8q[:, :, n0 : n0 + nw],
                              x8q_v[:, :, n0 : n0 + nw])
            nc.sync.dma_start(x8kv[:, :, n0 : n0 + nw],
                              xkv8_v[:, :, n0 : n0 + nw])

        def dma_seg_resid(si):
            # the bf16 residual is first read ~60us in; keep its transfer
            # out of the serialized dispatch queue ahead of the lead-in
            n0, nw = SEGS[si]
            nc.sync.dma_start(xq_f[:, :, n0 : n0 + nw],
                              xq_v[:, :, n0 : n0 + nw])

        def norms_seg(si):
            bag = smls.tile([P, 4, 2, 2], F32, tag="bag", name=f"bag{si}")
            for ci, (mi, m0, mw) in enumerate(_chunks_of_seg(si)):
                norms_for_chunk(mi, m0, mw, ci, bag)
                if si == 0 and ci == 0:
                    # chunk 0's rd alone unblocks the very first exp
                    rd_for_seg(0, bag, part=(0, 1))
            if si == 0:
                rd_for_seg(si, bag, part=(1, 4))
            else:
                rd_for_seg(si, bag)

        # ---------------- phase 2 ----------------
        er_tiles = {}

        def s_exp_chunk(sj, mi):
            sn0, snw = SUPERS[sj]
            m0, mw = M_CHUNKS[mi]
            sp = sS.tile([P, 2, SEG], F32, tag="sp", name=f"sp{sj}_{mi}")
            lhsT = k8 if use_bias else x8kv
            rhs = q8 if use_bias else g8
            nh = (snw + SEG - 1) // SEG
            for hi in range(nh):
                hw = min(SEG, snw - hi * SEG)
                nc.tensor.matmul(sp[:mw, hi, :hw], lhsT[:, :, m0 : m0 + mw],
                                 rhs[:, :, sn0 + hi * SEG : sn0 + hi * SEG + hw],
                                 start=True, stop=True, perf_mode=DR)
            pi, slot = mi // 2, mi % 2
            key = (sj, pi)
            if key not in er_tiles:
                if snw > SEG:
                    er_tiles[key] = e8p.tile([P, 2, 2, SEG], F8, tag="er",
                                             name=f"er{sj}_{pi}")
                else:
                    er_tiles[key] = e8s.tile([P, 2, 1, SEG], F8, tag="ers",
                                             name=f"er{sj}_{pi}")
            er = er_tiles[key]
            if snw > SEG:
                nc.scalar.activation(er[:mw, slot, :, :], sp[:mw, :, :], AF.Exp,
                                     scale=rd[:mw, mi : mi + 1])
            else:
                nc.scalar.activation(er[:mw, slot, 0, :snw], sp[:mw, 0, :snw],
                                     AF.Exp, scale=rd[:mw, mi : mi + 1])

        av_mid_q = []
        av_back_q = []

        def av_flush_back():
            while av_mid_q:
                av_mid_q.pop(0)()
            while av_back_q:
                av_back_q.pop(0)()

        def av_out_sub(sj, s):
                sn0, snw = SUPERS[sj]
                bw = min(P, snw - s * P)
                hh, c0 = s // 4, (s % 4) * P
                acc = aux.tile([P, SEG], F32, tag="aux", name=f"acc{sj}_{s}")
                for pi in range(12):
                    er = er_tiles[(sj, pi)]
                    nc.tensor.matmul(acc[:bw, : C + 2],
                                     er[:, :, hh, c0 : c0 + bw],
                                     pvT8[:, pi, :, :],
                                     start=(pi == 0), stop=False, perf_mode=DR)
                er = er_tiles[(sj, 12)]
                lmw = M_CHUNKS[24][1]
                nc.tensor.matmul(acc[:bw, : C + 2],
                                 er[:lmw, 0, hh, c0 : c0 + bw],
                                 pvT8[:lmw, 12, 0, :],
                                 start=False, stop=True)
                rc = smls.tile([P, 1], F32, tag="rc", name=f"rc{sj}_{s}")
                nc.vector.reciprocal(rc[:bw], acc[:bw, C : C + 1])
                un = scrp.tile([P, C], BF16, tag="un", name=f"un{sj}_{s}")
                nc.vector.tensor_scalar(un[:bw], acc[:bw, :C], rc[:bw], None,
                                        ALU.mult)
                pos = sn0 + s * P

                def mid(un=un, bw=bw, sj=sj, s=s, pos=pos):
                    # both c-chunks transpose into ONE psum tile (2nd matmul
                    # start=False accumulates into the already-zeroed region)
                    tp = aux.tile([P, 2, SEG], BF16, tag="aux",
                                  name=f"tp{sj}_{s}")
                    for cb in range(CC):
                        nc.tensor.matmul(tp[:, cb, :bw],
                                         un[:bw, cb * P : (cb + 1) * P],
                                         ident[:bw, :bw], is_transpose=True,
                                         start=(cb == 0), stop=(cb == CC - 1))

                    def back():
                        ot = outp.tile([P, CC, P], BF16, tag="ot",
                                       name=f"ot{sj}_{s}")
                        nc.vector.scalar_tensor_tensor(
                            ot[:, :, :bw], tp[:, :, :bw], 1.0 / WSCALE,
                            xq_f[:, :, pos : pos + bw], ALU.mult, ALU.add)
                        if use_bias:
                            for cb in range(CC):
                                nc.vector.tensor_scalar(ot[:, cb, :bw],
                                                        ot[:, cb, :bw],
                                                        bo_sb[:, cb : cb + 1],
                                                        None, ALU.add)
                        nc.sync.dma_start(out_v[:, :, pos : pos + bw],
                                          ot[:, :, :bw])

                    av_back_q.append(back)

                # stage the PE transposes one sub behind the AV matmuls and
                # the DVE output STT two behind, so neither engine's FIFO
                # ever stalls on a cross-engine round-trip
                av_mid_q.append(mid)
                if len(av_mid_q) > 1:
                    av_mid_q.pop(0)()
                if len(av_back_q) > 1:
                    av_back_q.pop(0)()

        def av_out_super(sj):
            snw = SUPERS[sj][1]
            for s in range((snw + P - 1) // P):
                av_out_sub(sj, s)

        # Work-queue emission: an exp for (super sj, chunk mi) is ready once
        # the q8 segs covering the super and the k8/rd seg covering the chunk
        # are computed. Emitting in availability order keeps the ACT queue
        # full from ~seg 2 onward. AV/output subtiles of completed supers are
        # interleaved between exps so the PE queue always has ready work.
        sup_ready_at = [(sn0 + snw - 1) // SEG for sn0, snw in SUPERS]
        n_chunks = len(M_CHUNKS)
        done_chunks = [set() for _ in SUPERS]
        av_pending = []
        av_done = 0
        FILL = 3

        def emit_av(k):
            nonlocal av_done
            while av_done < k and av_done < len(av_pending):
                av_out_sub(*av_pending[av_done])
                av_done += 1

        def emit_exp(sj, mi, av_rate=1):
            if mi in done_chunks[sj]:
                return
            s_exp_chunk(sj, mi)
            done_chunks[sj].add(mi)
            if len(done_chunks[sj]) == n_chunks:
                nsub = (SUPERS[sj][1] + P - 1) // P
                av_pending.extend((sj, s) for s in range(nsub))
            emit_av(av_done + av_rate)

        dma_seg(0)
        preamble()
        for si in range(len(SEGS)):
            if si + 1 < len(SEGS):
                dma_seg(si + 1)
            if si == 1:
                preamble_late()
            dma_seg_resid(si)
            n0, nw = SEGS[si]
            norms_seg(si)
            proj_for_seg(n0, nw, on_act=(si <= 1))
            for mi2 in PV_PLAN.get(si, ()):
                pv_for_chunk(mi2, *M_CHUNKS[mi2])
            avail = min(4 * (si + 1), n_chunks)
            # Once the last seg lands, the final chunk's exp gates EVERY
            # super's AV: emit all supers' chunk 24 first so AV work can
            # start executing while the remaining exps drain.
            if avail == n_chunks:
                for sj in range(len(SUPERS)):
                    emit_exp(sj, n_chunks - 1)
            # super 0 has priority: finish earlier supers first and fill ACT
            # with just a little of the next super to avoid gaps.
            if sup_ready_at[0] <= si:
                for mi in range(avail):
                    emit_exp(0, mi)
            if si >= 1:
                for sj in range(1, len(SUPERS)):
                    if sup_ready_at[sj] > si or len(done_chunks[sj]) >= avail:
                        continue
                    take = 0
                    for mi in range(avail):
                        if take >= FILL:
                            break
                        if mi not in done_chunks[sj]:
                            emit_exp(sj, mi)
                            take += 1
                    break
        # drain remaining supers, AV interleaved; the tiny last super is
        # drained second-to-last so the final super's exps cover its AV
        # Drain order: super 1, most of super 2, super 3, then the held-back
        # tail of super 2. Super 3's AV (gated by its last exp) then overlaps
        # the held-back window, and super 2's AV overlaps its own tail exps.
        for sj in range(1, len(SUPERS)):
            for mi in range(n_chunks):
                # the last super's exps are tiny (64 cols): pace AV slower
                # there so its S matmuls aren't buried behind AV matmuls
                rate = 1 if sj < len(SUPERS) - 1 else int(mi % 3 == 0)
                emit_exp(sj, mi, av_rate=rate)
        emit_av(len(av_pending))
        av_flush_back()

    return nc


_CACHE = {}


def _get_module(use_bias: bool):
    if use_bias not in _CACHE:
        nc = build(use_bias)
        nc.finalize()
        _CACHE[use_bias] = nc
    return _CACHE[use_bias]


def kernel(x_q, x_kv, Wq, bq, Wkv, bkv, Wproj, bproj):
    x_q = np.asarray(x_q, dtype=np.float32)
    x_kv = np.asarray(x_kv, dtype=np.float32)
    Wq = np.asarray(Wq, dtype=np.float32)
    bq = np.asarray(bq, dtype=np.float32)
    Wkv = np.asarray(Wkv, dtype=np.float32)
    bkv = np.asarray(bkv, dtype=np.float32)
    Wproj = np.asarray(Wproj, dtype=np.float32)
    bproj = np.asarray(bproj, dtype=np.float32)

    B, c, H, W = x_q.shape
    assert (c, H * W) == (C, N), (x_q.shape,)
    FP8 = ml_dtypes.float8_e4m3
    xq32 = x_q.reshape(B, C, N)
    xq = np.ascontiguousarray(xq32).astype(ml_dtypes.bfloat16)
    x8q = np.ascontiguousarray(xq32).astype(FP8)
    xkv8 = np.ascontiguousarray(x_kv.reshape(B, C, N)).astype(FP8)

    Wk = Wkv[:C]
    Wv = Wkv[C:]
    wq8 = np.ascontiguousarray(WSCALE * Wq.T).astype(FP8)
    wk8 = np.ascontiguousarray(WSCALE * Wk.T).astype(FP8)
    w38 = np.ascontiguousarray(WSCALE * (Wproj @ Wv).T).astype(FP8)
    gw8 = np.ascontiguousarray(WSCALE * (Wq.T @ Wk)).astype(FP8)
    bq16 = np.ascontiguousarray(WSCALE * bq)
    bk16 = np.ascontiguousarray(WSCALE * bkv[:C])
    bo = np.ascontiguousarray(Wproj @ bkv[C:] + bproj)

    use_bias = bool(np.any(bq16) or np.any(bk16) or np.any(bo))
    nc = _get_module(use_bias)

    in_maps = [
        {
            "xq": xq[b],
            "x8q": x8q[b],
            "xkv8": xkv8[b],
            "wq8": wq8,
            "wk8": wk8,
            "w38": w38,
            "gw8": gw8,
            "bq16": bq16,
            "bk16": bk16,
            "bo": bo,
        }
        for b in range(B)
    ]
    res = run_bass_kernel_spmd(nc, in_maps, core_ids=list(range(B)))
    out = np.stack([np.asarray(res.results[b]["out"]).astype(np.float32)
                    for b in range(B)], axis=0)
    return out.reshape(B, C, H, W)
